# revision 29
# baseline (speedup 1.0000x reference)
"""Trainium2 Bass kernel for a 3-layer edge-weighted GCN (graph message
passing), distributed over 8 NeuronCores.

Strategy (graph/data parallel, per the sharding hint):
  - Nodes are partitioned into 8 contiguous ranges of 6250; core c owns the
    edges whose dst lands in its range (so each core produces the final rows
    for its own node range with no cross-core reduction).
  - Algebraic reorder: reference computes segsum(w_e * (x@W)[src]); we
    compute segsum(w_e * x[src]) @ W  (exact, biases are zero per spec),
    so the dense matmul runs on the dst-sharded aggregate and only raw
    node features ever cross cores.
  - Aggregation on device: edges sorted by dst window (128 nodes); for each
    128-edge chunk, gather x[src] rows with dma_gather (SWDGE, 4 queues),
    and accumulate psum[f, n] += m[e, f]^T @ oh[e, n] on the TensorEngine,
    where oh is the w-scaled one-hot [edge, dst-offset] matrix PRECOMPUTED
    ON THE HOST and streamed densely from HBM (the DVE tensor_scalar build
    used ~440ns/instr of sequencer time and dominated the kernel).
  - Node features are stored/exchanged in bf16 (fp32 accumulation in PSUM);
    layer boundaries replicate the new features with an AllGather that is
    SLICED into 7 window-groups so transfers overlap the tail of the layer
    compute.  PSUM->SBUF copies ride the otherwise-idle Activation engine.
  - dma_gather indices are int16, so the 50176-row node table is split into
    two 25088-row halves; every (window, half) edge group is padded to whole
    128-edge chunks with w=0 edges, with a chunk count shared by all 8 cores
    so one SPMD program fits every core.
"""
import numpy as np
import ml_dtypes

import concourse.bass as bass
import concourse.bacc as bacc
import concourse.mybir as mybir
import concourse.tile as tile
from concourse.bass_utils import run_bass_kernel_spmd

# problem shape (hardcoded per spec nn_GCNModel_18073222381931)
N_NODES = 50000
N_EDGES = 500000
F = 128          # in feats
HID = 128        # hidden
OUT = 64         # classes
NCORES = 8

P = 128
NPC = N_NODES // NCORES            # 6250 nodes per core
NWIN = (NPC + P - 1) // P          # 49 windows of 128 dst nodes
NPAD = NWIN * P                    # 6272 padded nodes per core
NTOT = NCORES * NPAD               # 50176 padded node table rows
HALF = (NCORES // 2) * NPAD        # 25088 (int16-indexable halves)

GW = 8                             # windows per dma_gather batch
NSLICE = 1                         # collective slices per layer boundary
                                   # (measured: monolithic AllGather beats
                                   # sliced; each rendezvous costs ~20us)


def _new_pid(core, win, off, wps):
    """Slice-major node-table row: [slice, core, window-in-slice, off].
    Makes each sliced AllGather's output a contiguous table region."""
    return ((win // wps) * (NCORES * wps * P) + core * (wps * P)
            + (win % wps) * P + off)

bf16 = mybir.dt.bfloat16
f32 = mybir.dt.float32
bfnp = ml_dtypes.bfloat16


def _wrap_idx(idx_flat):
    """dma_gather index layout: edge i -> [i%16, i//16], replicated across
    the 8 Q7 partition groups."""
    n = len(idx_flat)
    assert n % 128 == 0
    w = idx_flat.reshape(n // 16, 16).T.astype(np.int16)   # [16, n//16]
    return np.ascontiguousarray(np.tile(w, (8, 1)))        # [128, n//16]


def prep(x, src, dst, w1, w2, w3, nslice=NSLICE):
    """Host-side sharding/index prep. Returns (structure, in_maps)."""
    src = np.asarray(src).astype(np.int64)
    dst = np.asarray(dst).astype(np.int64)
    ws = [np.asarray(w, np.float32) for w in (w1, w2, w3)]

    wps = -(-NWIN // nslice)               # windows per slice (padded)
    ntotp = NCORES * nslice * wps * P      # padded table rows
    halfp = ntotp // 2
    assert halfp < 2 ** 15
    npadp = nslice * wps * P               # hpart rows per core

    s_core = src // NPC
    s_loc = src % NPC
    src_pid = _new_pid(s_core, s_loc // P, s_loc % P, wps)
    core = dst // NPC
    loc = dst % NPC
    win = loc // P
    doff = (loc % P).astype(np.int64)
    half = (src_pid >= halfp).astype(np.int64)

    # chunk counts per (window, half), shared across cores (SPMD)
    cnt = np.zeros((NCORES, NWIN, 2), np.int64)
    np.add.at(cnt, (core, win, half), 1)
    nch = -(-cnt.max(axis=0) // P)                # [NWIN, 2] ceil
    for w in range(NWIN):
        if nch[w].sum() == 0:
            nch[w, 0] = 1
    ncha = int(nch[:, 0].sum())
    nchb = int(nch[:, 1].sum())
    ncht = ncha + nchb

    # global chunk index layout: per window, A chunks then B chunks
    chunk_base = np.zeros((NWIN, 2), np.int64)
    run = 0
    for w in range(NWIN):
        chunk_base[w, 0] = run
        run += nch[w, 0]
        chunk_base[w, 1] = run
        run += nch[w, 1]
    assert run == ncht
    epad = ncht * P

    # chunk -> half flag, and A/B-local chunk numbering
    chunk_half = np.zeros(ncht, np.int64)
    for w in range(NWIN):
        chunk_half[chunk_base[w, 1]:chunk_base[w, 1] + nch[w, 1]] = 1
    a_cols = np.nonzero(chunk_half == 0)[0]       # global chunk -> A list pos
    b_cols = np.nonzero(chunk_half == 1)[0]

    # per-core padded edge arrays in global chunk order
    gsrc = np.zeros((NCORES, epad), np.int64)     # padded node id (0 = pad)
    dofa = np.zeros((NCORES, epad), np.int64)
    wfa = np.zeros((3, NCORES, epad), np.float32)
    # pad entries in B chunks must index the B table: point at row halfp
    for w in range(NWIN):
        s = chunk_base[w, 1] * P
        e = s + nch[w, 1] * P
        gsrc[:, s:e] = halfp

    order = np.lexsort((half, win, core))
    so_core = core[order]
    so_win = win[order]
    so_half = half[order]
    so_src = src_pid[order]
    so_doff = doff[order]
    so_w = [w[order] for w in ws]
    # position within each (core, win, half) group
    keys = (so_core * NWIN * 2 + so_win * 2 + so_half)
    startmask = np.ones(len(keys), bool)
    startmask[1:] = keys[1:] != keys[:-1]
    gstart = np.nonzero(startmask)[0]
    within = np.arange(len(keys)) - np.repeat(
        gstart, np.diff(np.append(gstart, len(keys))))
    pos = chunk_base[so_win, so_half] * P + within
    gsrc[so_core, pos] = so_src
    dofa[so_core, pos] = so_doff
    for i in range(3):
        wfa[i, so_core, pos] = so_w[i]

    # host-precomputed w-scaled one-hot tables, streamed from HBM on device:
    # oh[l][p, g, n] = w_l[edge at (chunk g, lane p)] * (dstoff == n)
    lane = np.arange(epad) % P
    gch = np.arange(epad) // P
    ohs = np.zeros((3, NCORES, P, ncht, P), bfnp)
    for c in range(NCORES):
        col = dofa[c]
        for l in range(3):
            ohs[l, c][lane, gch, col] = wfa[l, c].astype(bfnp)

    xp = np.zeros((ntotp, F), bfnp)
    xf = np.asarray(x, np.float32)
    node = np.arange(N_NODES)
    n_loc = node % NPC
    nid = _new_pid(node // NPC, n_loc // P, n_loc % P, wps)
    xp[nid] = xf.astype(bfnp)

    in_maps = []
    for c in range(NCORES):
        idx_a = gsrc[c].reshape(ncht, P)[chunk_half == 0].ravel()
        idx_b = gsrc[c].reshape(ncht, P)[chunk_half == 1].ravel() - halfp
        if len(idx_b) == 0:
            idx_b = np.zeros(P, np.int64)
        in_maps.append({
            "xpA": np.ascontiguousarray(xp[:halfp]),
            "xpB": np.ascontiguousarray(xp[halfp:]),
            "idxA": _wrap_idx(idx_a),
            "idxB": _wrap_idx(idx_b),
            "oh1": np.ascontiguousarray(ohs[0, c]),
            "oh2": np.ascontiguousarray(ohs[1, c]),
            "oh3": np.ascontiguousarray(ohs[2, c]),
        })

    struct = {
        "nch": nch, "chunk_base": chunk_base, "ncha": ncha, "nchb": nchb,
        "ncht": ncht, "a_cols": a_cols, "b_cols": b_cols,
        "nslice": nslice, "wps": wps, "ntotp": ntotp, "halfp": halfp,
        "npadp": npadp,
    }
    return struct, in_maps


def build(struct, repeat=1, no_coll=False, no_gather=False,
          single_chunk=False, gather_queues=4, scratch=16384,
          single_packet=False, **_ignored):
    """Build the SPMD Bass program (weights arrive as ExternalInputs).

    The no_*/single_chunk flags build timing-ablation variants (numerically
    wrong); all default off for the real kernel."""
    nch = struct["nch"]
    ncha, nchb, ncht = struct["ncha"], struct["nchb"], struct["ncht"]
    nslice, wps = struct["nslice"], struct["wps"]
    ntotp, halfp, npadp = struct["ntotp"], struct["halfp"], struct["npadp"]
    a_of = {}
    b_of = {}
    # map global chunk -> position in A/B gather streams
    for i, g in enumerate(struct["a_cols"]):
        a_of[g] = i
    for i, g in enumerate(struct["b_cols"]):
        b_of[g] = i

    nc = bacc.Bacc("TRN2", target_bir_lowering=False, debug=False,
                   num_devices=NCORES, num_swdge_queues=gather_queues,
                   dynamic_dma_scratch_size=scratch)
    xpA = nc.dram_tensor("xpA", [halfp, F], bf16, kind="ExternalInput")
    xpB = nc.dram_tensor("xpB", [halfp, F], bf16, kind="ExternalInput")
    idxA = nc.dram_tensor("idxA", [P, ncha * 8], mybir.dt.int16,
                          kind="ExternalInput")
    idxB = nc.dram_tensor("idxB", [P, max(nchb, 1) * 8], mybir.dt.int16,
                          kind="ExternalInput")
    ohd = [nc.dram_tensor(f"oh{l+1}", [P, ncht, P], bf16,
                          kind="ExternalInput") for l in range(3)]
    Wd = [nc.dram_tensor("W1", [F, HID], bf16, kind="ExternalInput"),
          nc.dram_tensor("W2", [HID, HID], bf16, kind="ExternalInput"),
          nc.dram_tensor("W3", [HID, OUT], bf16, kind="ExternalInput")]
    out = nc.dram_tensor("out", [NPAD, OUT], f32, kind="ExternalOutput")
    hpart = [nc.dram_tensor(f"hpart{l}", [npadp, HID], bf16)
             for l in range(2)]
    hfull = [nc.dram_tensor(f"hfull{l}", [ntotp, HID], bf16,
                            addr_space="Shared") for l in range(2)]
    dummy = (nc.dram_tensor("dummy_gat", [P, 64 * F], bf16)
             if no_gather else None)

    with tile.TileContext(nc) as tc:
        with (
            tc.tile_pool(name="const", bufs=1) as cst,
            tc.tile_pool(name="ma", bufs=3) as map_,
            tc.tile_pool(name="mb", bufs=3) as mbp,
            tc.tile_pool(name="oh", bufs=4) as ohp,
            tc.tile_pool(name="agg", bufs=4) as aggp,
            tc.tile_pool(name="ho", bufs=4) as hop,
            tc.tile_pool(name="psa", bufs=4, space="PSUM") as psa,
            tc.tile_pool(name="pso", bufs=2, space="PSUM") as pso,
        ):
            idxA_sb = cst.tile([P, ncha * 8], mybir.dt.int16, tag="idxA")
            idxB_sb = cst.tile([P, max(nchb, 1) * 8], mybir.dt.int16,
                               tag="idxB")
            W_sb = [cst.tile([F, HID], bf16, tag="W1", name="W1_sb"),
                    cst.tile([HID, HID], bf16, tag="W2", name="W2_sb"),
                    cst.tile([HID, OUT], bf16, tag="W3", name="W3_sb")]
            nc.sync.dma_start(out=idxA_sb[:], in_=idxA[:, :])
            nc.sync.dma_start(out=idxB_sb[:], in_=idxB[:, :])
            for i in range(3):
                nc.sync.dma_start(out=W_sb[i][:], in_=Wd[i][:, :])

            wgroups = [list(range(g, min(g + GW, NWIN)))
                       for g in range(0, NWIN, GW)]
            qctr = [0]

            def next_q():
                q = qctr[0] % gather_queues
                qctr[0] += 1
                return q

            for _ in range(repeat):
                for l in range(3):
                    tabA = (xpA[:, :] if l == 0
                            else hfull[l - 1][0:halfp, :])
                    tabB = (xpB[:, :] if l == 0
                            else hfull[l - 1][halfp:ntotp, :])
                    outf = HID if l < 2 else OUT
                    for grp in wgroups:
                        ga = [struct["chunk_base"][w, 0] + i
                              for w in grp for i in range(nch[w, 0])]
                        gb = [struct["chunk_base"][w, 1] + i
                              for w in grp for i in range(nch[w, 1])]
                        kA, kB = len(ga), len(gb)
                        a0 = a_of[ga[0]] if kA else 0
                        b0 = b_of[gb[0]] if kB else 0
                        mAt = mBt = None
                        if kA:
                            mAt = map_.tile([P, kA, F], bf16, tag="mA")
                            if no_gather:
                                nc.sync.dma_start(out=mAt[:],
                                                  in_=dummy[:, :kA * F])
                            else:
                                nc.gpsimd.dma_gather(
                                    out_ap=mAt[:],
                                    in_ap=tabA,
                                    idxs_ap=idxA_sb[:, a0 * 8:(a0 + kA) * 8],
                                    num_idxs=kA * P,
                                    num_idxs_reg=kA * P,
                                    elem_size=F,
                                    single_packet=single_packet,
                                    queue_num=next_q(),
                                )
                        if kB:
                            mBt = mbp.tile([P, kB, F], bf16, tag="mB")
                            if no_gather:
                                nc.sync.dma_start(out=mBt[:],
                                                  in_=dummy[:, :kB * F])
                            else:
                                nc.gpsimd.dma_gather(
                                    out_ap=mBt[:],
                                    in_ap=tabB,
                                    idxs_ap=idxB_sb[:, b0 * 8:(b0 + kB) * 8],
                                    num_idxs=kB * P,
                                    num_idxs_reg=kB * P,
                                    elem_size=F,
                                    single_packet=single_packet,
                                    queue_num=next_q(),
                                )
                        for w in grp:
                            chunks = []
                            for i in range(nch[w, 0]):
                                g = struct["chunk_base"][w, 0] + i
                                chunks.append((mAt, a_of[g] - a0, g))
                            for i in range(nch[w, 1]):
                                g = struct["chunk_base"][w, 1] + i
                                chunks.append((mBt, b_of[g] - b0, g))
                            if single_chunk:
                                # keep one chunk per gather tile so neither
                                # gather is dead
                                keep = {}
                                sel = []
                                for ch in chunks:
                                    if id(ch[0]) not in keep:
                                        keep[id(ch[0])] = ch
                                        sel.append(ch)
                                chunks = sel
                            g0 = int(struct["chunk_base"][w, 0])
                            nchw = int(nch[w, 0] + nch[w, 1])
                            ohw = ohp.tile([P, nchw, P], bf16, tag="ohw")
                            nc.sync.dma_start(out=ohw[:],
                                              in_=ohd[l][:, g0:g0 + nchw, :])
                            pa = psa.tile([P, P], f32, tag="pa")
                            for j, (mt, lc, g) in enumerate(chunks):
                                nc.tensor.matmul(
                                    pa[:], lhsT=mt[:, lc, :],
                                    rhs=ohw[:, g - g0, :],
                                    start=(j == 0),
                                    stop=(j == len(chunks) - 1))
                            aggT = aggp.tile([P, P], bf16, tag="aggT")
                            nc.scalar.activation(
                                aggT[:], pa[:],
                                mybir.ActivationFunctionType.Copy)
                            po = pso.tile([P, outf], f32, tag="po")
                            nc.tensor.matmul(po[:], lhsT=aggT[:],
                                             rhs=W_sb[l][:, :],
                                             start=True, stop=True)
                            if l < 2:
                                ht = hop.tile([P, HID], bf16, tag="ht")
                                nc.scalar.activation(
                                    ht[:], po[:],
                                    mybir.ActivationFunctionType.Relu)
                                nc.sync.dma_start(
                                    out=hpart[l][w * P:(w + 1) * P, :],
                                    in_=ht[:])
                            else:
                                ot = hop.tile([P, OUT], f32, tag="ot")
                                nc.scalar.activation(
                                    ot[:], po[:],
                                    mybir.ActivationFunctionType.Copy)
                                nc.sync.dma_start(
                                    out=out[w * P:(w + 1) * P, :],
                                    in_=ot[:])
                    if l < 2 and not no_coll:
                        slrow = NCORES * wps * P
                        for s in range(nslice):
                            r0 = s * wps * P
                            r1 = (s + 1) * wps * P
                            nc.gpsimd.collective_compute(
                                "AllGather",
                                mybir.AluOpType.bypass,
                                replica_groups=[list(range(NCORES))],
                                ins=[hpart[l][r0:r1, :]],
                                outs=[hfull[l][s * slrow:(s + 1) * slrow, :]],
                            )
    nc.compile()
    return nc


def _to_in_maps(in_maps, Wmats):
    W1, W2, W3 = Wmats
    for m in in_maps:
        m["W1"] = np.asarray(W1, np.float32).astype(bfnp)
        m["W2"] = np.asarray(W2, np.float32).astype(bfnp)
        m["W3"] = np.asarray(W3, np.float32).astype(bfnp)
    return in_maps


def kernel(x, src, dst, w1, w2, w3, W1, b1, W2, b2, W3, b3, _repeat=1,
           _prebuilt=None):
    # biases are zero by construction (spec fill=zeros)
    if np.any(b1) or np.any(b2) or np.any(b3):
        raise NotImplementedError("nonzero biases not supported")
    struct, in_maps = prep(x, src, dst, w1, w2, w3)
    in_maps = _to_in_maps(in_maps, (W1, W2, W3))
    nc = _prebuilt or build(struct, repeat=_repeat)
    res = run_bass_kernel_spmd(nc, in_maps, list(range(NCORES)))
    outs = [res.results[c]["out"][:NPC] for c in range(NCORES)]
    return np.concatenate(outs, axis=0).astype(np.float32)


if __name__ == "__main__":
    pass
